# revision 26
# baseline (speedup 1.0000x reference)
"""LiteLinear (dense linear + per-token LoRA adapters) on 8 Trainium2 cores.

Sharding: data-parallel over tokens. Each core computes 1024 tokens:
  out = x @ W^T + bias + per-token LoRA delta.

Device kernel (per core), all matmul operands in bfloat16 (PSUM
accumulation stays fp32, as does the bias add and the output):
  - Computes out^T [D_OUT x TOK]; host transposes back on assembly.
  - Stationary operand = weight sub-chunk [128d x 128o], moving = x^T
    [128d x 512tok]. x^T resident in SBUF (64KB/partition in bf16).
  - bf16 weights enable Fast Weight Load (fp32/fp32r does not): the
    per-matmul LDWEIGHTS drops from ~226ns to ~53ns and hides fully
    behind the 213ns matmul stream, which is what moves the kernel from
    ~284ns/MM sustained to the ~213ns/MM streaming floor.
  - A_cat^T (the concatenated LoRA down-projections) is prepended to W^T
    as a 33rd output column tile, so h^T = A_cat @ x^T rides the same
    streamed matmul pipeline; its eviction is a DVE multiply with a
    host-built maskT (folds scalings + one-hot) producing hmask^T (bf16).
  - The combined [A|W]^T stream is re-laid-out on the host in quad-major
    form: one dma_start per 4 contraction chunks, 2KB contiguous lines,
    issued on the scalar HWDGE ring; x^T chunks alternate between the
    sync and gpsimd rings so arrival keeps ahead of consumption.
  - o-groups of [4] + [2]*14 + [1] x128 tiles (33 total; group 0 holds
    the A tile and is wide enough to consume x slower than it arrives).
    Width-2 groups use 4 PSUM banks each, so two groups pipeline through
    the 8 banks and boundary evictions overlap the next group's matmuls.
  - Per-token LoRA delta enters each out-tile as one extra accumulating
    matmul (lhsT=B_cat chunk, rhs=hmask^T); group 0 evicts the h tile
    (producing hmask) before issuing its own deltas.
  - Bias folded into PSUM->SBUF eviction via per-partition
    tensor_scalar_add; one batched output DMA per group on the sync ring.
"""

import numpy as np
import ml_dtypes

import sys

if "/opt/trn_rl_repo" not in sys.path:
    sys.path.insert(0, "/opt/trn_rl_repo")

import concourse.bass as bass
import concourse.mybir as mybir
import concourse.tile as tile
from concourse import bacc
from concourse.bass_utils import run_bass_kernel_spmd

N_TOK = 8192
D_IN = 4096
D_OUT = 4096
N_ADAPTERS = 8
RANK = 16
AR = N_ADAPTERS * RANK  # 128
N_CORES = 8
TOK = N_TOK // N_CORES  # 1024 tokens per core

P = 128            # partitions
FREE = 512         # matmul moving free dim (fp32 max, == 1 PSUM bank)
KC = D_IN // P     # 32 contraction chunks
KQ = 4             # k-chunks per quad DMA
NQ = KC // KQ      # 8 quads
TH = TOK // FREE   # 2 token halves
NO = D_OUT // P + 1  # 33 o128-tiles incl. the A tile (index 0)
# Group 0 is width 4 so its k-loop consumes x chunks slower than they
# arrive from HBM (no startup stall); later groups are width 2 so each
# uses 4 PSUM banks and two groups pipeline through the 8 banks, letting
# group-boundary evictions overlap the next group's MMs.
GROUPS = [4] + [2] * 14 + [1]  # o128-tiles per group (sum 33)

F32 = mybir.dt.float32
BF16 = mybir.dt.bfloat16
NP_BF16 = ml_dtypes.bfloat16

_CACHE = {}


def _build_nc():
    nc = bacc.Bacc(None, target_bir_lowering=False, debug=True)

    xT = nc.dram_tensor("xT", [D_IN, TOK], BF16, kind="ExternalInput")
    # quad-major [A|W]: [kq, p, (g kk cols_g)] with per-group contiguous blocks
    wTr = nc.dram_tensor("wTr", [NQ, P, KQ * NO * P], BF16,
                         kind="ExternalInput")
    bcat = nc.dram_tensor("bcat", [AR, D_OUT], BF16, kind="ExternalInput")
    maskT = nc.dram_tensor("maskT", [AR, TOK], F32, kind="ExternalInput")
    biasr = nc.dram_tensor("biasr", [P, D_OUT // P], F32, kind="ExternalInput")
    outT = nc.dram_tensor("outT", [D_OUT, TOK], F32, kind="ExternalOutput")

    def w_quad_src(kq, goff, blk, sub_off=0):
        return bass.AP(
            tensor=wTr[:].tensor,
            offset=kq * P * KQ * NO * P + goff + sub_off,
            ap=[[KQ * NO * P, P], [1, blk]],
        )

    with tile.TileContext(nc) as tc:
        with (
            tc.tile_pool(name="xpool", bufs=1) as xpool,
            tc.tile_pool(name="const", bufs=1) as const,
            tc.tile_pool(name="wpool", bufs=4) as wpool,
            tc.tile_pool(name="opool", bufs=3) as opool,
            tc.tile_pool(name="psum", bufs=8, space="PSUM") as psum,
        ):
            hmask = const.tile([P, TOK], BF16, tag="hmask")
            biasr_sb = const.tile([P, D_OUT // P], F32, tag="biasr")
            maskT_sb = const.tile([P, TOK], F32, tag="maskT")
            bcat_sb = const.tile([P, D_OUT], BF16, tag="bcat")

            xt = []

            def base_loop(g, width, goff, pg, startup):
                """32 k-chunks of base matmuls for one o-group."""
                wt = None
                for k in range(KC):
                    if startup:
                        t = xpool.tile([P, TOK], BF16, tag=f"xt{k}",
                                       name=f"xt{k}")
                        if k == 0:
                            # split so the very first matmul only waits on
                            # the first 512-token half
                            nc.sync.dma_start(out=t[:, :FREE],
                                              in_=xT[0:P, :FREE])
                            nc.sync.dma_start(out=t[:, FREE:],
                                              in_=xT[0:P, FREE:])
                        else:
                            eng = nc.sync if k % 2 == 0 else nc.gpsimd
                            eng.dma_start(out=t[:],
                                          in_=xT[k * P:(k + 1) * P, :])
                        xt.append(t)
                    if k % KQ == 0:
                        wt = wpool.tile([P, KQ * width * P], BF16, tag="wt",
                                        name=f"wt{g}_{k}")
                        if startup and k == 0:
                            # per-k pieces so the k=0 matmuls start early
                            for kk2 in range(KQ):
                                nc.scalar.dma_start(
                                    out=wt[:, kk2 * width * P:
                                           (kk2 + 1) * width * P],
                                    in_=w_quad_src(0, goff, width * P,
                                                   sub_off=kk2 * width * P))
                        else:
                            nc.scalar.dma_start(
                                out=wt[:],
                                in_=w_quad_src(k // KQ, goff, KQ * width * P))
                    if startup and k == 17:
                        nc.sync.dma_start(out=biasr_sb[:], in_=biasr[:, :])
                        nc.sync.dma_start(out=maskT_sb[:], in_=maskT[:, :])
                    if startup and k == 21:
                        nc.sync.dma_start(out=bcat_sb[:], in_=bcat[:, :])
                    kk = k % KQ
                    for j in range(width):
                        for th in range(TH):
                            tsl = slice(th * FREE, (th + 1) * FREE)
                            mm = nc.tensor.matmul(
                                pg[j * TH + th][:],
                                wt[:, (kk * width + j) * P:
                                   (kk * width + j + 1) * P],
                                xt[k][:, tsl],
                                start=(k == 0),
                                stop=(k == KC - 1 and g == 0 and j == 0),
                            )
                            if th > 0:
                                # hint: same stationary as th=0 (the lowering
                                # currently still emits a LDWEIGHTS, which FWL
                                # hides behind the 213ns matmul stream)
                                mm.ldweights = False

            def flush(g, width, ooff, pg):
                """Delta matmuls + bias evictions + out DMA for group g."""
                j0 = 1 if g == 0 else 0
                nreal = width - j0
                ob = opool.tile([P, nreal * TOK], F32, tag="ob",
                                name=f"ob_{g}")
                for j in range(j0, width):
                    om = ooff + j - 1  # real W o128-tile index
                    jb = j - j0
                    for th in range(TH):
                        tsl = slice(th * FREE, (th + 1) * FREE)
                        mm = nc.tensor.matmul(
                            pg[j * TH + th][:],
                            bcat_sb[:, om * P:(om + 1) * P],
                            hmask[:, tsl],
                            start=False, stop=True,
                        )
                        if th > 0:
                            mm.ldweights = False
                    for th in range(TH):
                        tsl = slice(jb * TOK + th * FREE,
                                    jb * TOK + (th + 1) * FREE)
                        nc.vector.tensor_scalar_add(
                            ob[:, tsl], pg[j * TH + th][:],
                            biasr_sb[:, om:om + 1],
                        )
                om0 = ooff + j0 - 1
                nc.sync.dma_start(
                    out=bass.AP(
                        tensor=outT[:].tensor,
                        offset=om0 * P * TOK,
                        ap=[[TOK, P], [P * TOK, nreal], [1, TOK]],
                    ),
                    in_=ob[:],
                )

            ooff = 0  # in o128-tiles over the combined [A|W] column space
            for g, width in enumerate(GROUPS):
                pg = [
                    psum.tile([P, FREE], F32, tag="ps", name=f"pg{g}_{i}")
                    for i in range(width * TH)
                ]
                base_loop(g, width, KQ * ooff * P, pg, startup=(g == 0))
                if g == 0:
                    # evict the A tile -> hmask (scaled, masked) on DVE.
                    # The A tile's accumulation ends at k=KC-1, j=0, so the
                    # DVE muls overlap the remaining j=1..3 base matmuls.
                    for th in range(TH):
                        tsl = slice(th * FREE, (th + 1) * FREE)
                        nc.vector.tensor_mul(
                            hmask[:, tsl], pg[th][:], maskT_sb[:, tsl])
                flush(g, width, ooff, pg)
                ooff += width

    nc.compile()
    return nc


def _prep_inputs(x, weight, bias, lora_a, lora_b, scalings, lora_mapping):
    x = np.ascontiguousarray(x, dtype=np.float32)
    weight = np.ascontiguousarray(weight, dtype=np.float32)
    bias = np.ascontiguousarray(bias, dtype=np.float32)
    lora_a = np.ascontiguousarray(lora_a, dtype=np.float32)
    lora_b = np.ascontiguousarray(lora_b, dtype=np.float32)
    scalings = np.ascontiguousarray(scalings, dtype=np.float32)
    lora_mapping = np.asarray(lora_mapping)

    xT = np.ascontiguousarray(x.T.astype(NP_BF16))                   # [D_IN, N_TOK]
    aT = lora_a.transpose(2, 0, 1).reshape(D_IN, AR)                 # [D_IN,(a r)]
    awT = np.concatenate([aT, weight.T], axis=1).astype(NP_BF16)     # [D_IN, NO*P]
    # quad-major [A|W] with per-group contiguous (kk, cols) blocks
    w4 = awT.reshape(NQ, KQ, P, NO * P)                              # [kq,kk,p,o]
    blocks = []
    o0 = 0
    for wdt in GROUPS:
        blk = w4[:, :, :, o0:o0 + wdt * P]                           # [kq,kk,p,w]
        blocks.append(blk.transpose(0, 2, 1, 3).reshape(NQ, P, KQ * wdt * P))
        o0 += wdt * P
    wTr = np.ascontiguousarray(np.concatenate(blocks, axis=2))

    bcat = np.ascontiguousarray(
        lora_b.transpose(0, 2, 1).reshape(AR, D_OUT).astype(NP_BF16))  # [(a r), D_OUT]
    # biasr[p, m] = bias[m*128 + p]
    biasr = np.ascontiguousarray(bias.reshape(D_OUT // P, P).T)      # [P, 32]
    # maskT[(a r), n] = scalings[a] * (lora_mapping[n] == a+1)
    ids = np.arange(1, N_ADAPTERS + 1, dtype=lora_mapping.dtype)
    onehot = (lora_mapping[None, :] == ids[:, None]).astype(np.float32)  # [A, N]
    maskT = (onehot * scalings[:, None]).repeat(RANK, axis=0)        # [(a r), N]
    maskT = np.ascontiguousarray(maskT)

    in_maps = []
    for c in range(N_CORES):
        tsl = slice(c * TOK, (c + 1) * TOK)
        in_maps.append({
            "xT": np.ascontiguousarray(xT[:, tsl]),
            "wTr": wTr,
            "bcat": bcat,
            "maskT": np.ascontiguousarray(maskT[:, tsl]),
            "biasr": biasr,
        })
    return in_maps


def run(inputs, trace=False):
    if "nc" not in _CACHE:
        _CACHE["nc"] = _build_nc()
    nc = _CACHE["nc"]
    in_maps = _prep_inputs(**inputs)
    res = run_bass_kernel_spmd(
        nc, in_maps, list(range(N_CORES)), trace=trace,
    )
    out = np.concatenate(
        [np.ascontiguousarray(r["outT"].T) for r in res.results], axis=0
    )
    return out, res


def kernel(**inputs) -> np.ndarray:
    out, _ = run(inputs, trace=False)
    return out



# revision 27
# speedup vs baseline: 1.0058x; 1.0058x over previous
"""LiteLinear (dense linear + per-token LoRA adapters) on 8 Trainium2 cores.

Sharding: data-parallel over tokens. Each core computes 1024 tokens:
  out = x @ W^T + bias + per-token LoRA delta.

Device kernel (per core), all matmul operands in bfloat16 (PSUM
accumulation stays fp32, as does the bias add and the output):
  - Computes out^T [D_OUT x TOK]; host transposes back on assembly.
  - Stationary operand = weight sub-chunk [128d x 128o], moving = x^T
    [128d x 512tok]. x^T resident in SBUF (64KB/partition in bf16).
  - bf16 weights enable Fast Weight Load (fp32/fp32r does not): the
    per-matmul LDWEIGHTS drops from ~226ns to ~53ns and hides fully
    behind the 213ns matmul stream, which is what moves the kernel from
    ~284ns/MM sustained to the ~213ns/MM streaming floor.
  - A_cat^T (the concatenated LoRA down-projections) is prepended to W^T
    as a 33rd output column tile, so h^T = A_cat @ x^T rides the same
    streamed matmul pipeline; its eviction is a DVE multiply with a
    host-built maskT (folds scalings + one-hot) producing hmask^T (bf16).
  - The combined [A|W]^T stream is re-laid-out on the host in quad-major
    form: one dma_start per 4 contraction chunks, 2KB contiguous lines,
    issued on the scalar HWDGE ring; x^T chunks alternate between the
    sync and gpsimd rings so arrival keeps ahead of consumption.
  - o-groups of [4] + [2]*14 + [1] x128 tiles (33 total; group 0 holds
    the A tile and is wide enough to consume x slower than it arrives).
    Width-2 groups use 4 PSUM banks each, so two groups pipeline through
    the 8 banks and boundary evictions overlap the next group's matmuls.
  - Per-token LoRA delta enters each out-tile as one extra accumulating
    matmul (lhsT=B_cat chunk, rhs=hmask^T); group 0 evicts the h tile
    (producing hmask) before issuing its own deltas.
  - Bias folded into PSUM->SBUF eviction via per-partition
    tensor_scalar_add; one batched output DMA per group on the sync ring.
"""

import numpy as np
import ml_dtypes

import sys

if "/opt/trn_rl_repo" not in sys.path:
    sys.path.insert(0, "/opt/trn_rl_repo")

import concourse.bass as bass
import concourse.mybir as mybir
import concourse.tile as tile
from concourse import bacc
from concourse.bass_utils import run_bass_kernel_spmd

N_TOK = 8192
D_IN = 4096
D_OUT = 4096
N_ADAPTERS = 8
RANK = 16
AR = N_ADAPTERS * RANK  # 128
N_CORES = 8
TOK = N_TOK // N_CORES  # 1024 tokens per core

P = 128            # partitions
FREE = 512         # matmul moving free dim (fp32 max, == 1 PSUM bank)
KC = D_IN // P     # 32 contraction chunks
KQ = 4             # k-chunks per quad DMA
NQ = KC // KQ      # 8 quads
TH = TOK // FREE   # 2 token halves
NO = D_OUT // P + 1  # 33 o128-tiles incl. the A tile (index 0)
# Group 0 is width 4 so its k-loop consumes x chunks slower than they
# arrive from HBM (no startup stall); later groups are width 2 so each
# uses 4 PSUM banks and two groups pipeline through the 8 banks, letting
# group-boundary evictions overlap the next group's MMs.
GROUPS = [4] + [2] * 14 + [1]  # o128-tiles per group (sum 33)

F32 = mybir.dt.float32
BF16 = mybir.dt.bfloat16
NP_BF16 = ml_dtypes.bfloat16

_CACHE = {}


def _build_nc():
    nc = bacc.Bacc(None, target_bir_lowering=False, debug=True)

    xT = nc.dram_tensor("xT", [D_IN, TOK], BF16, kind="ExternalInput")
    # quad-major [A|W]: [kq, p, (g kk cols_g)] with per-group contiguous blocks
    wTr = nc.dram_tensor("wTr", [NQ, P, KQ * NO * P], BF16,
                         kind="ExternalInput")
    bcat = nc.dram_tensor("bcat", [AR, D_OUT], BF16, kind="ExternalInput")
    maskT = nc.dram_tensor("maskT", [AR, TOK], F32, kind="ExternalInput")
    biasr = nc.dram_tensor("biasr", [P, D_OUT // P], F32, kind="ExternalInput")
    outT = nc.dram_tensor("outT", [D_OUT, TOK], F32, kind="ExternalOutput")

    def w_quad_src(kq, goff, blk, sub_off=0):
        return bass.AP(
            tensor=wTr[:].tensor,
            offset=kq * P * KQ * NO * P + goff + sub_off,
            ap=[[KQ * NO * P, P], [1, blk]],
        )

    with tile.TileContext(nc) as tc:
        with (
            tc.tile_pool(name="xpool", bufs=1) as xpool,
            tc.tile_pool(name="const", bufs=1) as const,
            tc.tile_pool(name="wpool", bufs=4) as wpool,
            tc.tile_pool(name="opool", bufs=3) as opool,
            tc.tile_pool(name="psum", bufs=8, space="PSUM") as psum,
        ):
            hmask = const.tile([P, TOK], BF16, tag="hmask")
            biasr_sb = const.tile([P, D_OUT // P], F32, tag="biasr")
            maskT_sb = const.tile([P, TOK], F32, tag="maskT")
            bcat_sb = const.tile([P, D_OUT], BF16, tag="bcat")

            xt = []

            def base_loop(g, width, goff, pg, startup):
                """32 k-chunks of base matmuls for one o-group."""
                wt = None
                for k in range(KC):
                    if startup:
                        t = xpool.tile([P, TOK], BF16, tag=f"xt{k}",
                                       name=f"xt{k}")
                        eng = nc.sync if k % 2 == 0 else nc.gpsimd
                        eng.dma_start(out=t[:], in_=xT[k * P:(k + 1) * P, :])
                        xt.append(t)
                    if k % KQ == 0:
                        wt = wpool.tile([P, KQ * width * P], BF16, tag="wt",
                                        name=f"wt{g}_{k}")
                        nc.scalar.dma_start(
                            out=wt[:],
                            in_=w_quad_src(k // KQ, goff, KQ * width * P))
                    if startup and k == 17:
                        nc.sync.dma_start(out=biasr_sb[:], in_=biasr[:, :])
                        nc.sync.dma_start(out=maskT_sb[:], in_=maskT[:, :])
                    if startup and k == 21:
                        nc.sync.dma_start(out=bcat_sb[:], in_=bcat[:, :])
                    kk = k % KQ
                    for j in range(width):
                        for th in range(TH):
                            tsl = slice(th * FREE, (th + 1) * FREE)
                            mm = nc.tensor.matmul(
                                pg[j * TH + th][:],
                                wt[:, (kk * width + j) * P:
                                   (kk * width + j + 1) * P],
                                xt[k][:, tsl],
                                start=(k == 0),
                                stop=(k == KC - 1 and g == 0 and j == 0),
                            )
                            if th > 0:
                                # hint: same stationary as th=0 (the lowering
                                # currently still emits a LDWEIGHTS, which FWL
                                # hides behind the 213ns matmul stream)
                                mm.ldweights = False

            def flush(g, width, ooff, pg):
                """Delta matmuls + bias evictions + out DMA for group g."""
                j0 = 1 if g == 0 else 0
                nreal = width - j0
                ob = opool.tile([P, nreal * TOK], F32, tag="ob",
                                name=f"ob_{g}")
                for j in range(j0, width):
                    om = ooff + j - 1  # real W o128-tile index
                    jb = j - j0
                    for th in range(TH):
                        tsl = slice(th * FREE, (th + 1) * FREE)
                        mm = nc.tensor.matmul(
                            pg[j * TH + th][:],
                            bcat_sb[:, om * P:(om + 1) * P],
                            hmask[:, tsl],
                            start=False, stop=True,
                        )
                        if th > 0:
                            mm.ldweights = False
                    for th in range(TH):
                        tsl = slice(jb * TOK + th * FREE,
                                    jb * TOK + (th + 1) * FREE)
                        nc.vector.tensor_scalar_add(
                            ob[:, tsl], pg[j * TH + th][:],
                            biasr_sb[:, om:om + 1],
                        )
                om0 = ooff + j0 - 1
                nc.sync.dma_start(
                    out=bass.AP(
                        tensor=outT[:].tensor,
                        offset=om0 * P * TOK,
                        ap=[[TOK, P], [P * TOK, nreal], [1, TOK]],
                    ),
                    in_=ob[:],
                )

            ooff = 0  # in o128-tiles over the combined [A|W] column space
            for g, width in enumerate(GROUPS):
                pg = [
                    psum.tile([P, FREE], F32, tag="ps", name=f"pg{g}_{i}")
                    for i in range(width * TH)
                ]
                base_loop(g, width, KQ * ooff * P, pg, startup=(g == 0))
                if g == 0:
                    # evict the A tile -> hmask (scaled, masked) on DVE.
                    # The A tile's accumulation ends at k=KC-1, j=0, so the
                    # DVE muls overlap the remaining j=1..3 base matmuls.
                    for th in range(TH):
                        tsl = slice(th * FREE, (th + 1) * FREE)
                        nc.vector.tensor_mul(
                            hmask[:, tsl], pg[th][:], maskT_sb[:, tsl])
                flush(g, width, ooff, pg)
                ooff += width

    nc.compile()
    return nc


def _prep_inputs(x, weight, bias, lora_a, lora_b, scalings, lora_mapping):
    x = np.ascontiguousarray(x, dtype=np.float32)
    weight = np.ascontiguousarray(weight, dtype=np.float32)
    bias = np.ascontiguousarray(bias, dtype=np.float32)
    lora_a = np.ascontiguousarray(lora_a, dtype=np.float32)
    lora_b = np.ascontiguousarray(lora_b, dtype=np.float32)
    scalings = np.ascontiguousarray(scalings, dtype=np.float32)
    lora_mapping = np.asarray(lora_mapping)

    xT = np.ascontiguousarray(x.T.astype(NP_BF16))                   # [D_IN, N_TOK]
    aT = lora_a.transpose(2, 0, 1).reshape(D_IN, AR)                 # [D_IN,(a r)]
    awT = np.concatenate([aT, weight.T], axis=1).astype(NP_BF16)     # [D_IN, NO*P]
    # quad-major [A|W] with per-group contiguous (kk, cols) blocks
    w4 = awT.reshape(NQ, KQ, P, NO * P)                              # [kq,kk,p,o]
    blocks = []
    o0 = 0
    for wdt in GROUPS:
        blk = w4[:, :, :, o0:o0 + wdt * P]                           # [kq,kk,p,w]
        blocks.append(blk.transpose(0, 2, 1, 3).reshape(NQ, P, KQ * wdt * P))
        o0 += wdt * P
    wTr = np.ascontiguousarray(np.concatenate(blocks, axis=2))

    bcat = np.ascontiguousarray(
        lora_b.transpose(0, 2, 1).reshape(AR, D_OUT).astype(NP_BF16))  # [(a r), D_OUT]
    # biasr[p, m] = bias[m*128 + p]
    biasr = np.ascontiguousarray(bias.reshape(D_OUT // P, P).T)      # [P, 32]
    # maskT[(a r), n] = scalings[a] * (lora_mapping[n] == a+1)
    ids = np.arange(1, N_ADAPTERS + 1, dtype=lora_mapping.dtype)
    onehot = (lora_mapping[None, :] == ids[:, None]).astype(np.float32)  # [A, N]
    maskT = (onehot * scalings[:, None]).repeat(RANK, axis=0)        # [(a r), N]
    maskT = np.ascontiguousarray(maskT)

    in_maps = []
    for c in range(N_CORES):
        tsl = slice(c * TOK, (c + 1) * TOK)
        in_maps.append({
            "xT": np.ascontiguousarray(xT[:, tsl]),
            "wTr": wTr,
            "bcat": bcat,
            "maskT": np.ascontiguousarray(maskT[:, tsl]),
            "biasr": biasr,
        })
    return in_maps


def run(inputs, trace=False):
    if "nc" not in _CACHE:
        _CACHE["nc"] = _build_nc()
    nc = _CACHE["nc"]
    in_maps = _prep_inputs(**inputs)
    res = run_bass_kernel_spmd(
        nc, in_maps, list(range(N_CORES)), trace=trace,
    )
    out = np.concatenate(
        [np.ascontiguousarray(r["outT"].T) for r in res.results], axis=0
    )
    return out, res


def kernel(**inputs) -> np.ndarray:
    out, _ = run(inputs, trace=False)
    return out

